# revision 19
# baseline (speedup 1.0000x reference)
"""Trainium2 Bass kernel for nn_PhysicsLoss (PINN physics loss).

Computes, for a 3->256->256->256->256->4 tanh MLP f:
    J = jac(f), Hdiag = diag-of-hessian(f) per point, then
    continuity/momentum residuals -> phy_loss [N].

Strategy (per core, data-parallel over 8 cores):
  - Feature-major layout: activations [256 feats = 2x128 partitions, pts free].
  - Forward-mode propagation of 5 streams per hidden layer:
      h (forward), d_i = g*t_i (3 Jacobian tangents, seeded W0[i,:]/x_std[i]),
      E = summed/weighted 2nd-order stream (only the Laplacian is needed).
    Per layer: t_i/new S via matmul; g = 1-h^2; d_i = g*T_i;
      Q = 2*sum_i T_i^2; E' = g*(S - h*Q).
  - Output layer: residuals r = sum_s A_s^T stream_s with host-precomputed
    A_s [256,4] folding W4, y_std, 1/x_std, nu and the residual wiring.
  - loss = sum_y r_y^2 via ACT Square + ones-matmul.
"""

import numpy as np
from contextlib import ExitStack

import concourse.bass as bass
import concourse.bacc as bacc
import concourse.tile as tile
from concourse import mybir
from concourse.bass_utils import run_bass_kernel_spmd

N_FULL = 32768
NCORES = 8
NPC = N_FULL // NCORES  # 4096 points per core
B = 512                 # points per tile
TILES = NPC // B
H = 256
P = 128

F32 = mybir.dt.float32
BF16 = mybir.dt.bfloat16
F16 = mybir.dt.float16
AF = mybir.ActivationFunctionType
OP = mybir.AluOpType
SQRT2 = float(np.sqrt(2.0))

_build_cache = {}


def bcast3(ap, n):
    """Insert a broadcast (step-0) middle free dim of size n into a [P, B] AP."""
    return bass.AP(tensor=ap.tensor, offset=ap.offset, ap=[ap.ap[0], [0, n], ap.ap[1]])


def build_v2(dt_mm="f16", spool_bufs=None, tpool_bufs=None):
    import os
    if spool_bufs is None:
        spool_bufs = int(os.environ.get("SPOOL_BUFS", "3"))
    if tpool_bufs is None:
        tpool_bufs = int(os.environ.get("TPOOL_BUFS", "3"))
    """fp16 pipeline v4: fp16 streams; L0 seeds folded into layer-1 weights
    (layer-1 matmul rhs are h, g, c=g*h); early-freed S bank (gS = g*S);
    T3 psum double-buffered for PE overlap."""
    dt_s = {"f16": F16, "bf16v2": BF16}[dt_mm]

    nc = bacc.Bacc("TRN2", target_bir_lowering=False, debug=False, num_devices=NCORES)
    xT_d = nc.dram_tensor("xT", [4, NPC], dt_s, kind="ExternalInput").ap()
    w0_d = nc.dram_tensor("w0", [4, H], dt_s, kind="ExternalInput").ap()
    wpk_d = nc.dram_tensor("wpk", [P, 28 * P], dt_s, kind="ExternalInput").ap()
    apk_d = nc.dram_tensor("apk", [P, 32], dt_s, kind="ExternalInput").ap()
    vec_d = nc.dram_tensor("vec", [P, 12], F32, kind="ExternalInput").ap()
    out_d = nc.dram_tensor("out", [1, NPC], F32, kind="ExternalOutput").ap()

    with tile.TileContext(nc) as tc, ExitStack() as ctx:
        const = ctx.enter_context(tc.tile_pool(name="const", bufs=1))
        spool = ctx.enter_context(tc.tile_pool(name="streams", bufs=spool_bufs))
        tpool = ctx.enter_context(tc.tile_pool(name="tmp", bufs=tpool_bufs))
        psZ = ctx.enter_context(tc.tile_pool(name="psZ", bufs=1, space="PSUM"))
        psT = ctx.enter_context(tc.tile_pool(name="psT", bufs=2, space="PSUM"))
        psS = ctx.enter_context(tc.tile_pool(name="psS", bufs=1, space="PSUM"))

        xT_sb = const.tile([4, NPC], dt_s)
        nc.sync.dma_start(xT_sb[:], xT_d)
        w0_sb = const.tile([4, H], dt_s)
        nc.sync.dma_start(w0_sb[:], w0_d)
        wpk_sb = const.tile([P, 28 * P], dt_s)
        nc.sync.dma_start(wpk_sb[:], wpk_d)
        apk_sb = const.tile([P, 32], dt_s)
        nc.sync.dma_start(apk_sb[:], apk_d)
        vec_sb = const.tile([P, 12], F32)
        nc.sync.dma_start(vec_sb[:], vec_d)
        out_sb = const.tile([1, NPC], F32)
        ones_sb = const.tile([4, 1], dt_s)
        nc.vector.memset(ones_sb[:], 1.0)

        def vcol(m, j):  # bias col j for chunk m
            return vec_sb[:, m * 4 + j : m * 4 + j + 1]

        def wblk(idx):
            return wpk_sb[:, idx * P : (idx + 1) * P]

        for t in range(TILES):
            ts = slice(t * B, (t + 1) * B)
            # ---- Layer 0: h, g, c = g*h  (seeds live in layer-1 weights) ----
            h = spool.tile([P, 2, B], dt_s, tag="h")
            g = tpool.tile([P, 2, B], dt_s, tag="g")
            c = tpool.tile([P, 2, B], dt_s, tag="c")
            hh = tpool.tile([P, 2, B], dt_s, tag="hh")
            for m in range(2):
                z = psZ.tile([P, B], F32, tag="Zh")
                nc.tensor.matmul(
                    z[:], w0_sb[:, m * P : (m + 1) * P], xT_sb[:, ts],
                    start=True, stop=True,
                )
                nc.scalar.activation(h[:, m, :], z[:], AF.Tanh, bias=vcol(m, 0))
            nc.scalar.activation(hh[:], h[:], AF.Square)
            nc.scalar.activation(g[:], hh[:], AF.Identity, bias=1.0, scale=-1.0)
            nc.vector.tensor_tensor(c[:], g[:], h[:], OP.mult)
            D, E = None, None

            # ---- Hidden layers 1..3 ----
            for l in range(3):
                hN = spool.tile([P, 2, B], dt_s, tag="h")
                DN = spool.tile([P, 3, 2, B], dt_s, tag="D")
                EN = spool.tile([P, 2, B], dt_s, tag="E")
                gN = tpool.tile([P, 2, B], dt_s, tag="g")
                cN = tpool.tile([P, 2, B], dt_s, tag="c")
                hh = tpool.tile([P, 2, B], dt_s, tag="hh")
                SQ = tpool.tile([P, 2, 3, B], dt_s, tag="SQ")
                qA = tpool.tile([P, 2, B], dt_s, tag="qA")
                Q = tpool.tile([P, 2, B], dt_s, tag="Q")
                u = tpool.tile([P, 2, B], dt_s, tag="u")
                gS = tpool.tile([P, 2, B], dt_s, tag="gS")
                T3s = []
                for m in range(2):
                    zh = psZ.tile([P, B], F32, tag="Zh")
                    T3 = psT.tile([P, 3, B], F32, tag="T3")
                    pS = psS.tile([P, B], F32, tag="S")
                    T3s.append(T3)
                    for k in range(2):
                        if l == 0:
                            wz = wblk(k * 2 + m)
                            wt = [wblk(12 + i * 4 + k * 2 + m) for i in range(3)]
                            ws = wblk(24 + k * 2 + m)
                            rz, rt, rs = h[:, k, :], [g[:, k, :]] * 3, c[:, k, :]
                        else:
                            wz = wblk(l * 4 + k * 2 + m)
                            wt = [wblk(l * 4 + k * 2 + m)] * 3
                            ws = wblk(l * 4 + k * 2 + m)
                            rz = h[:, k, :]
                            rt = [D[:, i, k, :] for i in range(3)]
                            rs = E[:, k, :]
                        nc.tensor.matmul(
                            zh[:], wz, rz, start=(k == 0), stop=(k == 1)
                        )
                        for i in range(3):
                            nc.tensor.matmul(
                                T3[:, i, :], wt[i], rt[i],
                                start=(k == 0), stop=(k == 1),
                            )
                        nc.tensor.matmul(
                            pS[:], ws, rs, start=(k == 0), stop=(k == 1)
                        )
                    nc.scalar.activation(hN[:, m, :], zh[:], AF.Tanh, bias=vcol(m, 1 + l))
                    T3s.append(pS)
                nc.scalar.activation(hh[:], hN[:], AF.Square)
                nc.scalar.activation(gN[:], hh[:], AF.Identity, bias=1.0, scale=-1.0)
                for m in range(2):
                    nc.vector.tensor_tensor(
                        DN[:, :, m, :], bcast3(gN[:, m, :], 3), T3s[2 * m][:], OP.mult
                    )
                    nc.scalar.activation(
                        SQ[:, m, :, :], T3s[2 * m][:], AF.Square, scale=SQRT2
                    )
                for m in range(2):
                    nc.vector.tensor_tensor(
                        gS[:, m, :], gN[:, m, :], T3s[2 * m + 1][:], OP.mult
                    )
                nc.vector.tensor_tensor(qA[:], SQ[:, :, 0, :], SQ[:, :, 1, :], OP.add)
                nc.vector.tensor_tensor(Q[:], qA[:], SQ[:, :, 2, :], OP.add)
                nc.vector.tensor_tensor(cN[:], gN[:], hN[:], OP.mult)
                nc.vector.tensor_tensor(u[:], cN[:], Q[:], OP.mult)
                nc.vector.tensor_tensor(EN[:], gS[:], u[:], OP.subtract)
                h, D, E, g, c = hN, DN, EN, gN, cN

            # ---- Output layer ----
            r = psZ.tile([4, B], F32, tag="Zh")
            idx = 0
            for s in range(4):
                for k in range(2):
                    rhs = D[:, s, k, :] if s < 3 else E[:, k, :]
                    nc.tensor.matmul(
                        r[:], apk_sb[:, (s * 2 + k) * 4 : (s * 2 + k + 1) * 4], rhs,
                        start=(idx == 0), stop=(idx == 7),
                    )
                    idx += 1
            rsq = tpool.tile([4, B], dt_s, tag="rsq")
            nc.scalar.activation(rsq[:], r[:], AF.Square)
            lt = psS.tile([1, B], F32, tag="S")
            nc.tensor.matmul(lt[:], ones_sb[:], rsq[:], start=True, stop=True)
            nc.scalar.copy(out_sb[0:1, ts], lt[:])
        nc.sync.dma_start(out_d[:], out_sb[:])

    nc.compile()
    return nc


def build_module(dt_mm="f32"):
    """Build (and compile) the per-core Bass module.

    dt_mm in {f32, f32r, bf16} (v1 pipeline) or {f16, bf16v2} (v2 pipeline).
    """
    key = dt_mm
    if key in _build_cache:
        return _build_cache[key]
    if dt_mm in ("f16", "bf16v2"):
        import os
        key = (dt_mm, os.environ.get("SPOOL_BUFS", "3"), os.environ.get("TPOOL_BUFS", "3"))
        if key in _build_cache:
            return _build_cache[key]
        nc = build_v2(dt_mm)
        _build_cache[key] = nc
        return nc

    # stream dtype in SBUF (matmul inputs)
    dt_s = BF16 if dt_mm == "bf16" else F32

    nc = bacc.Bacc(
        "TRN2", target_bir_lowering=False, debug=False, num_devices=NCORES
    )
    dt_io = BF16 if dt_mm == "bf16" else F32
    xT_d = nc.dram_tensor("xT", [4, NPC], dt_io, kind="ExternalInput").ap()
    w0_d = nc.dram_tensor("w0", [4, H], dt_io, kind="ExternalInput").ap()
    wpk_d = nc.dram_tensor("wpk", [P, 12 * P], dt_io, kind="ExternalInput").ap()
    apk_d = nc.dram_tensor("apk", [P, 32], dt_io, kind="ExternalInput").ap()
    vec_d = nc.dram_tensor("vec", [P, 20], F32, kind="ExternalInput").ap()
    out_d = nc.dram_tensor("out", [1, NPC], F32, kind="ExternalOutput").ap()

    def mmt(ap):
        # view an f32 AP under the matmul dtype (f32r = full-rate fp32 mode)
        if dt_mm == "f32r":
            return ap.bitcast(mybir.dt.float32r)
        return ap

    with tile.TileContext(nc) as tc, ExitStack() as ctx:
        const = ctx.enter_context(tc.tile_pool(name="const", bufs=1))
        spool = ctx.enter_context(tc.tile_pool(name="streams", bufs=spool_bufs))
        tpool = ctx.enter_context(tc.tile_pool(name="tmp", bufs=2))
        psZ = ctx.enter_context(tc.tile_pool(name="psZ", bufs=2, space="PSUM"))
        psT = ctx.enter_context(tc.tile_pool(name="psT", bufs=1, space="PSUM"))
        psR = ctx.enter_context(tc.tile_pool(name="psR", bufs=1, space="PSUM"))

        xT_sb = const.tile([4, NPC], dt_s)
        nc.sync.dma_start(xT_sb[:], xT_d)
        w0_sb = const.tile([4, H], dt_s)
        nc.sync.dma_start(w0_sb[:], w0_d)
        wpk_sb = const.tile([P, 12 * P], dt_s)
        nc.sync.dma_start(wpk_sb[:], wpk_d)
        apk_sb = const.tile([P, 32], dt_s)
        nc.sync.dma_start(apk_sb[:], apk_d)
        vec_sb = const.tile([P, 20], F32)
        nc.sync.dma_start(vec_sb[:], vec_d)
        out_sb = const.tile([1, NPC], F32)
        if dt_mm == "bf16":
            ones_sb = const.tile([4, 1], dt_s)
            nc.vector.memset(ones_sb[:], 1.0)

        def vcol(m, j):
            return vec_sb[:, m * 8 + j : m * 8 + j + 1]

        for t in range(TILES):
            ts = slice(t * B, (t + 1) * B)
            # ---- Layer 0 ----
            h = spool.tile([P, 2, B], dt_s, tag="h")
            d = [spool.tile([P, 2, B], dt_s, name=f"d{i}", tag=f"d{i}") for i in range(3)]
            E = spool.tile([P, 2, B], dt_s, tag="E")
            for m in range(2):
                z = psZ.tile([P, B], F32, tag="Zh")
                nc.tensor.matmul(
                    z[:],
                    mmt(w0_sb[:, m * P : (m + 1) * P]),
                    mmt(xT_sb[:, ts]),
                    start=True,
                    stop=True,
                )
                nc.scalar.activation(h[:, m, :], z[:], AF.Tanh, bias=vcol(m, 0))
                hh = tpool.tile([P, B], F32, tag="hh")
                nc.scalar.activation(hh[:], h[:, m, :], AF.Square)
                g = tpool.tile([P, B], F32, tag="g")
                nc.scalar.activation(g[:], hh[:], AF.Identity, bias=1.0, scale=-1.0)
                for i in range(3):
                    nc.vector.tensor_scalar(
                        d[i][:, m, :], g[:], vcol(m, 4 + i), None, OP.mult
                    )
                t1 = tpool.tile([P, B], F32, tag="t1")
                nc.vector.tensor_scalar(t1[:], h[:, m, :], vcol(m, 7), None, OP.mult)
                nc.vector.tensor_tensor(E[:, m, :], g[:], t1[:], OP.mult)

            # ---- Hidden layers 1..3 ----
            for l in range(3):
                hN = spool.tile([P, 2, B], dt_s, tag="h")
                dN = [spool.tile([P, 2, B], dt_s, name=f"dN{i}", tag=f"d{i}") for i in range(3)]
                EN = spool.tile([P, 2, B], dt_s, tag="E")
                for m in range(2):
                    zh = psZ.tile([P, B], F32, tag="Zh")
                    pT = [psT.tile([P, B], F32, name=f"T{i}", tag=f"T{i}") for i in range(3)]
                    pS = psT.tile([P, B], F32, tag="S")
                    outs = [zh, pT[0], pT[1], pT[2], pS]
                    srcs = [h, d[0], d[1], d[2], E]
                    for k in range(2):
                        w_ap = wpk_sb[:, (l * 4 + k * 2 + m) * P : (l * 4 + k * 2 + m + 1) * P]
                        for s in range(5):
                            nc.tensor.matmul(
                                outs[s][:],
                                mmt(w_ap),
                                mmt(srcs[s][:, k, :]),
                                start=(k == 0),
                                stop=(k == 1),
                            )
                    nc.scalar.activation(
                        hN[:, m, :], zh[:], AF.Tanh, bias=vcol(m, 1 + l)
                    )
                    hh = tpool.tile([P, B], F32, tag="hh")
                    nc.scalar.activation(hh[:], hN[:, m, :], AF.Square)
                    g = tpool.tile([P, B], F32, tag="g")
                    nc.scalar.activation(
                        g[:], hh[:], AF.Identity, bias=1.0, scale=-1.0
                    )
                    for i in range(3):
                        nc.vector.tensor_tensor(
                            dN[i][:, m, :], g[:], pT[i][:], OP.mult
                        )
                    sq = [tpool.tile([P, B], F32, name=f"sq{i}", tag=f"sq{i}") for i in range(3)]
                    for i in range(3):
                        nc.scalar.activation(sq[i][:], pT[i][:], AF.Square, scale=SQRT2)
                    qA = tpool.tile([P, B], F32, tag="qA")
                    nc.vector.tensor_tensor(qA[:], sq[0][:], sq[1][:], OP.add)
                    Q = tpool.tile([P, B], F32, tag="Q")
                    nc.vector.tensor_tensor(Q[:], qA[:], sq[2][:], OP.add)
                    t1 = tpool.tile([P, B], F32, tag="t1")
                    nc.vector.tensor_tensor(t1[:], hN[:, m, :], Q[:], OP.mult)
                    t2 = tpool.tile([P, B], F32, tag="t2")
                    nc.vector.tensor_tensor(t2[:], pS[:], t1[:], OP.subtract)
                    nc.vector.tensor_tensor(EN[:, m, :], g[:], t2[:], OP.mult)
                h, d, E = hN, dN, EN

            # ---- Output layer: residuals r[4, B] ----
            r = psR.tile([4, B], F32, tag="r")
            srcs = [d[0], d[1], d[2], E]
            idx = 0
            for s in range(4):
                for k in range(2):
                    nc.tensor.matmul(
                        r[:],
                        mmt(apk_sb[:, (s * 2 + k) * 4 : (s * 2 + k + 1) * 4]),
                        mmt(srcs[s][:, k, :]),
                        start=(idx == 0),
                        stop=(idx == 7),
                    )
                    idx += 1
            rsq = tpool.tile([4, B], dt_s, tag="rsq")
            nc.scalar.activation(rsq[:], r[:], AF.Square)
            lt = psR.tile([1, B], F32, tag="lt")
            if dt_mm == "bf16":
                ones_ap = ones_sb[:]
            else:
                ones_ap = vec_sb[0:4, 16:17]
            nc.tensor.matmul(lt[:], mmt(ones_ap), mmt(rsq[:]), start=True, stop=True)
            nc.vector.tensor_copy(out_sb[0:1, ts], lt[:])
        nc.sync.dma_start(out_d[:], out_sb[:])

    nc.compile()
    _build_cache[key] = nc
    return nc


def host_prep(inputs, dt_mm="f32"):
    """Precompute per-core input maps (numpy only)."""
    x = np.ascontiguousarray(np.asarray(inputs["x_norm"], np.float32))
    nu = float(np.asarray(inputs["nu"]).reshape(-1)[0])
    xs = np.asarray(inputs["x_std"], np.float32)
    ys = np.asarray(inputs["y_std"], np.float32)
    W0 = np.asarray(inputs["W0"], np.float32)
    b = [np.asarray(inputs[f"b{i}"], np.float32) for i in range(4)]
    Ws = [np.asarray(inputs[f"W{i}"], np.float32) for i in range(1, 4)]
    W4 = np.asarray(inputs["W4"], np.float32)

    wp = [W0[i, :] / xs[i] for i in range(3)]
    q0 = (-2.0 * sum(w * w for w in wp)).astype(np.float32)
    W4y = (W4 * ys[None, :]).astype(np.float32)
    A = np.zeros((4, H, 4), np.float32)
    A[0][:, 0] = W4y[:, 0]
    A[0][:, 1] = W4y[:, 3]
    A[1][:, 0] = W4y[:, 1]
    A[1][:, 2] = W4y[:, 3]
    A[2][:, 0] = W4y[:, 2]
    A[2][:, 3] = W4y[:, 3]
    A[3][:, 1] = -nu * W4y[:, 0]
    A[3][:, 2] = -nu * W4y[:, 1]
    A[3][:, 3] = -nu * W4y[:, 2]

    w0_np = np.zeros((4, H), np.float32)
    w0_np[0:3, :] = W0

    v2 = dt_mm in ("f16", "bf16v2")
    nblk = 28 if v2 else 12
    wpk_np = np.zeros((P, nblk * P), np.float32)
    for l in range(3):
        for k in range(2):
            for m in range(2):
                idx = l * 4 + k * 2 + m
                wpk_np[:, idx * P : (idx + 1) * P] = Ws[l][
                    k * P : (k + 1) * P, m * P : (m + 1) * P
                ]
    if v2:
        W1wi = [Ws[0] * wp[i][:, None] for i in range(3)]
        W1q = Ws[0] * q0[:, None]
        for i in range(3):
            for k in range(2):
                for m in range(2):
                    idx = 12 + i * 4 + k * 2 + m
                    wpk_np[:, idx * P : (idx + 1) * P] = W1wi[i][
                        k * P : (k + 1) * P, m * P : (m + 1) * P
                    ]
        for k in range(2):
            for m in range(2):
                idx = 24 + k * 2 + m
                wpk_np[:, idx * P : (idx + 1) * P] = W1q[
                    k * P : (k + 1) * P, m * P : (m + 1) * P
                ]

    apk_np = np.zeros((P, 32), np.float32)
    for s in range(4):
        for k in range(2):
            apk_np[:, (s * 2 + k) * 4 : (s * 2 + k + 1) * 4] = A[s][
                k * P : (k + 1) * P, :
            ]

    if v2:
        vec_np = np.zeros((P, 12), np.float32)
        for m in range(2):
            sl = slice(m * P, (m + 1) * P)
            for j in range(4):
                vec_np[:, m * 4 + j] = b[j][sl]
    else:
        vec_np = np.zeros((P, 20), np.float32)
        for m in range(2):
            sl = slice(m * P, (m + 1) * P)
            for j in range(4):
                vec_np[:, m * 8 + j] = b[j][sl]
            for i in range(3):
                vec_np[:, m * 8 + 4 + i] = wp[i][sl]
            vec_np[:, m * 8 + 7] = q0[sl]
        vec_np[0:4, 16] = 1.0

    if dt_mm in ("bf16", "bf16v2"):
        import ml_dtypes

        def cast(a):
            return a.astype(ml_dtypes.bfloat16)
    elif dt_mm == "f16":
        def cast(a):
            return a.astype(np.float16)
    else:
        def cast(a):
            return a

    in_maps = []
    for c in range(NCORES):
        xc = x[c * NPC : (c + 1) * NPC, :]  # [NPC, 3]
        xT = np.zeros((4, NPC), np.float32)
        xT[0:3, :] = xc.T
        in_maps.append(
            {
                "xT": np.ascontiguousarray(cast(xT)),
                "w0": np.ascontiguousarray(cast(w0_np)),
                "wpk": np.ascontiguousarray(cast(wpk_np)),
                "apk": np.ascontiguousarray(cast(apk_np)),
                "vec": vec_np,
            }
        )
    return in_maps


def run(inputs, dt_mm="f32", trace=False):
    nc = build_module(dt_mm)
    in_maps = host_prep(inputs, dt_mm)
    res = run_bass_kernel_spmd(nc, in_maps, list(range(NCORES)), trace=trace)
    out = np.concatenate([res.results[c]["out"].reshape(-1) for c in range(NCORES)])
    return out.astype(np.float32), res


def kernel(**inputs):
    out, _ = run(inputs, dt_mm="f16")
    return out
